# revision 9
# baseline (speedup 1.0000x reference)
"""ControlNorm2DLoop Trainium2 kernel.

x: [64, 256, 64, 64] f32. Per-(n,c) spatial moments over (H,W), then a
sequential EMA over the batch dim updates per-channel (m, v); each sample is
normalized with the state *before* its update.

Strategy: shard C across 8 cores (32 ch/core). Single pass over x in
supertiles of SPT samples -> SBUF tiles [128 = 4*32 partitions, NQ*4096],
where each quarter (4 samples x 32 channels) fills the 128 partitions and
quarters sit side by side in the free dim. Stats via bn_stats/bn_aggr (DVE).
The EMA recurrence is linear, so the within-quarter state propagation is a
constant triangular matrix applied on the TensorEngine (contracts over
partitions); the cross-quarter carry is a replicated [128,1] state tile
updated with elementwise DVE ops. Normalization is done in-place on the x
tile by the scalar engine (Identity(x*scale+bias) with per-partition
scale/bias). Loads are issued on the SP HWDGE ring, stores on the ACT HWDGE
ring.

PE wait discipline: walrus allows only ONE sync-wait command on a
(self-loading fp32) Matmult, so all constants arrive in a single DMA that a
warmup matmul observes once, and everything else a matmul touches (rhs
vectors, recycled PSUM slots) is produced/consumed exclusively by the DVE.
"""

import sys

if "/opt/trn_rl_repo" not in sys.path:
    sys.path.insert(0, "/opt/trn_rl_repo")

from contextlib import ExitStack

import numpy as np

AFWD = 0.999
EPS = 1e-05
N, C, H, W = 64, 256, 64, 64
NCORES = 8
CSH = C // NCORES     # 32 channels per core
G = 4                 # samples per quarter (fills 128 partitions)
FD = H * W            # 4096
P = G * CSH           # 128 partitions

SPT = 8               # samples per supertile
NQ = SPT // G         # quarters per supertile
NT = N // SPT         # supertiles per core
XBUFS = 4             # supertile double/triple buffering

# packed const layout (columns of the [128, 513] const tile)
COL_SCAN_M = 0
COL_SCAN_V = 128
COL_TAIL_M = 256
COL_TAIL_V = 384
COL_APOW = 512
CONST_COLS = 513


def _build_const() -> np.ndarray:
    """One [128, 513] f32 tile holding all scan matrices + A^s column.

    m_vals[(s,c)] = sum_{t<s} (1-A)A^(s-1-t) mu[(t,c)] + A^s m_state[c]
    v_vals[(s,c)] = sum_{t<s} (1-A)A^(s-1-t) w'[(t,c)] + A^s v_state[c]
      with w' = var + A*(mu-m)^2  (the (1-A) lives in the matrices)
    state'[c] = sum_t (1-A)A^(G-1-t) mu[(t,c)] + A^G state[c]
      (tail matrices replicate state' across all 4 sample slots)
    """
    A = AFWD
    k = np.zeros((P, CONST_COLS), np.float32)
    for s in range(G):
        for t in range(s):
            coef = (1 - A) * A ** (s - 1 - t)
            for c in range(CSH):
                k[t * CSH + c, COL_SCAN_M + s * CSH + c] = coef
                k[t * CSH + c, COL_SCAN_V + s * CSH + c] = coef
    for t in range(G):
        coef = (1 - A) * A ** (G - 1 - t)
        for s in range(G):
            for c in range(CSH):
                k[t * CSH + c, COL_TAIL_M + s * CSH + c] = coef
                k[t * CSH + c, COL_TAIL_V + s * CSH + c] = coef
    for s in range(G):
        k[s * CSH:(s + 1) * CSH, COL_APOW] = A ** s
    return k


_CACHE = {}


def build_nc():
    """Build (and cache) the Bass program. Same program for all 8 cores."""
    if "nc" in _CACHE:
        return _CACHE["nc"]

    import concourse.bacc as bacc
    import concourse.tile as tile
    from concourse import mybir

    f32 = mybir.dt.float32
    Alu = mybir.AluOpType
    Act = mybir.ActivationFunctionType
    AG = AFWD ** G

    nc = bacc.Bacc()
    x_d = nc.declare_dram_parameter("x", [N * CSH, FD], f32, isOutput=False)
    const_d = nc.declare_dram_parameter("consts", [P, CONST_COLS], f32,
                                        isOutput=False)
    out_d = nc.declare_dram_parameter("out", [N * CSH, FD], f32, isOutput=True)

    with tile.TileContext(nc) as tc, ExitStack() as ctx:
        const = ctx.enter_context(tc.tile_pool(name="const", bufs=1))
        xp = ctx.enter_context(tc.tile_pool(name="xp", bufs=XBUFS))
        st = ctx.enter_context(tc.tile_pool(name="st", bufs=3))
        states = ctx.enter_context(tc.tile_pool(name="states", bufs=2))
        psA = ctx.enter_context(tc.tile_pool(name="psA", bufs=2, space="PSUM"))
        psB = ctx.enter_context(tc.tile_pool(name="psB", bufs=1, space="PSUM"))

        ct = const.tile([P, CONST_COLS], f32)
        nc.sync.dma_start(out=ct, in_=const_d[:])
        lhs_scan_m = ct[:, COL_SCAN_M:COL_SCAN_M + P]
        lhs_scan_v = ct[:, COL_SCAN_V:COL_SCAN_V + P]
        lhs_tail_m = ct[:, COL_TAIL_M:COL_TAIL_M + P]
        lhs_tail_v = ct[:, COL_TAIL_V:COL_TAIL_V + P]
        apow = ct[:, COL_APOW:COL_APOW + 1]

        # PE touches the const tile once, so later matmuls carry no DMA wait.
        warm = psB.tile([P, 1], f32)
        nc.tensor.matmul(warm, lhsT=lhs_scan_m, rhs=apow, start=True, stop=True)

        # replicated per-(s,c) carry state: every sample slot holds state[c]
        m_rep = states.tile([P, 1], f32)
        nc.vector.memset(m_rep, 0.0)
        v_rep = states.tile([P, 1], f32)
        nc.vector.memset(v_rep, 1.0)

        for g in range(NT):
            xt = xp.tile([P, NQ * FD], f32)
            rows = slice(g * NQ * P, (g + 1) * NQ * P)
            nc.sync.dma_start(
                out=xt.rearrange("p (h f) -> p h f", h=NQ),
                in_=x_d[rows, :].rearrange("(h p) f -> p h f", p=P),
            )

            for q in range(NQ):
                xq = xt[:, q * FD:(q + 1) * FD]

                # per-(sample,channel) mean/var over the 4096 free elements
                bnst = st.tile([P, FD // 512, 6], f32)
                xq_chunks = xq.rearrange("p (k f) -> p k f", f=512)
                for k in range(FD // 512):
                    nc.vector.bn_stats(out=bnst[:, k, :], in_=xq_chunks[:, k, :])
                mv = st.tile([P, 2], f32)
                nc.vector.bn_aggr(out=mv, in_=bnst)
                mu = mv[:, 0:1]
                var = mv[:, 1:2]

                # m_vals[(s,c)] = m_{n0+s,c}: triangular on PE, carry on DVE
                pm = psA.tile([P, 1], f32)
                nc.tensor.matmul(pm, lhsT=lhs_scan_m, rhs=mu, start=True,
                                 stop=True)
                pmrep = psB.tile([P, 1], f32)
                nc.tensor.matmul(pmrep, lhsT=lhs_tail_m, rhs=mu, start=True,
                                 stop=True)
                mc = st.tile([P, 1], f32)
                nc.vector.tensor_tensor(out=mc, in0=apow, in1=m_rep,
                                        op=Alu.mult)
                m_neg = st.tile([P, 1], f32)
                nc.vector.scalar_tensor_tensor(
                    out=m_neg, in0=pm, scalar=-1.0, in1=mc,
                    op0=Alu.mult, op1=Alu.subtract,
                )  # -(pm + A^s*state)

                # w' = var + A*(mu - m)^2
                d = st.tile([P, 1], f32)
                nc.vector.tensor_tensor(out=d, in0=mu, in1=m_neg, op=Alu.add)
                d2 = st.tile([P, 1], f32)
                nc.vector.tensor_tensor(out=d2, in0=d, in1=d, op=Alu.mult)
                wp = st.tile([P, 1], f32)
                nc.vector.scalar_tensor_tensor(
                    out=wp, in0=d2, scalar=AFWD, in1=var,
                    op0=Alu.mult, op1=Alu.add,
                )

                # v_vals + eps, assembled straight into SBUF
                pv = psA.tile([P, 1], f32)
                nc.tensor.matmul(pv, lhsT=lhs_scan_v, rhs=wp, start=True,
                                 stop=True)
                pvrep = psB.tile([P, 1], f32)
                nc.tensor.matmul(pvrep, lhsT=lhs_tail_v, rhs=wp, start=True,
                                 stop=True)
                vc = st.tile([P, 1], f32)
                nc.vector.tensor_tensor(out=vc, in0=apow, in1=v_rep,
                                        op=Alu.mult)
                ve = st.tile([P, 1], f32)
                nc.vector.scalar_tensor_tensor(
                    out=ve, in0=pv, scalar=EPS, in1=vc,
                    op0=Alu.add, op1=Alu.add,
                )  # pv + eps + A^s*v_state

                # next-quarter replicated states (serial chain)
                new_m = states.tile([P, 1], f32)
                nc.vector.scalar_tensor_tensor(
                    out=new_m, in0=m_rep, scalar=AG, in1=pmrep,
                    op0=Alu.mult, op1=Alu.add,
                )
                m_rep = new_m
                new_v = states.tile([P, 1], f32)
                nc.vector.scalar_tensor_tensor(
                    out=new_v, in0=v_rep, scalar=AG, in1=pvrep,
                    op0=Alu.mult, op1=Alu.add,
                )
                v_rep = new_v

                # scale = 1/sqrt(v + eps); bias = -m * scale
                s0 = st.tile([P, 1], f32)
                nc.scalar.activation(out=s0, in_=ve, func=Act.Sqrt)
                sc = st.tile([P, 1], f32)
                nc.vector.reciprocal(out=sc, in_=s0)
                b = st.tile([P, 1], f32)
                nc.vector.tensor_scalar(
                    out=b, in0=m_neg, scalar1=sc, scalar2=None, op0=Alu.mult
                )

                # out = x*scale + bias, in place
                nc.scalar.activation(
                    out=xq, in_=xq, func=Act.Identity, bias=b, scale=sc
                )

            # store the whole supertile on the ACT HWDGE ring
            nc.scalar.dma_start(
                out=out_d[rows, :].rearrange("(h p) f -> p h f", p=P),
                in_=xt.rearrange("p (h f) -> p h f", h=NQ),
            )

    nc.compile()
    _CACHE["nc"] = nc
    return nc


def kernel(x: np.ndarray) -> np.ndarray:
    assert x.shape == (N, C, H, W) and x.dtype == np.float32
    nc = build_nc()
    from concourse.bass_utils import run_bass_kernel_spmd

    consts = _build_const()
    in_maps = []
    for k in range(NCORES):
        shard = np.ascontiguousarray(
            x[:, k * CSH:(k + 1) * CSH]
        ).reshape(N * CSH, FD)
        in_maps.append({"x": shard, "consts": consts})

    res = run_bass_kernel_spmd(nc, in_maps, core_ids=list(range(NCORES)))
    shards = [res.results[k]["out"].reshape(N, CSH, H, W) for k in range(NCORES)]
    return np.concatenate(shards, axis=1)


# revision 11
# speedup vs baseline: 26.3638x; 26.3638x over previous
"""ControlNorm2DLoop Trainium2 kernel.

x: [64, 256, 64, 64] f32. Per-(n,c) spatial moments over (H,W), then a
sequential EMA over the batch dim updates per-channel (m, v); each sample is
normalized with the state *before* its update.

Strategy: shard C across 8 cores (32 ch/core). Single pass over x in
supertiles of SPT samples -> SBUF tiles [128 = 4*32 partitions, NQ*4096],
where each quarter (4 samples x 32 channels) fills the 128 partitions and
quarters sit side by side in the free dim. Stats via bn_stats/bn_aggr (DVE).
The EMA recurrence is linear, so the within-quarter state propagation is a
constant triangular matrix applied on the TensorEngine (contracts over
partitions); the cross-quarter carry is a replicated [128,1] state tile
updated with elementwise DVE ops. Normalization is done in-place on the x
tile by the scalar engine (Identity(x*scale+bias) with per-partition
scale/bias). Loads are issued on the SP HWDGE ring, stores on the ACT HWDGE
ring.

PE wait discipline: walrus allows only ONE sync-wait command on a
(self-loading fp32) Matmult, so all constants arrive in a single DMA that a
warmup matmul observes once, and everything else a matmul touches (rhs
vectors, recycled PSUM slots) is produced/consumed exclusively by the DVE.
"""

import sys

if "/opt/trn_rl_repo" not in sys.path:
    sys.path.insert(0, "/opt/trn_rl_repo")

from contextlib import ExitStack

import numpy as np

AFWD = 0.999
EPS = 1e-05
N, C, H, W = 64, 256, 64, 64
NCORES = 8
CSH = C // NCORES     # 32 channels per core
G = 4                 # samples per quarter (fills 128 partitions)
FD = H * W            # 4096
P = G * CSH           # 128 partitions

SPT = 8               # samples per supertile
NQ = SPT // G         # quarters per supertile
NT = N // SPT         # supertiles per core
XBUFS = 5             # supertile double/triple buffering

# packed const layout (columns of the [128, 513] const tile)
COL_SCAN_M = 0
COL_SCAN_V = 128
COL_TAIL_M = 256
COL_TAIL_V = 384
COL_APOW = 512
CONST_COLS = 513


def _build_const() -> np.ndarray:
    """One [128, 513] f32 tile holding all scan matrices + A^s column.

    m_vals[(s,c)] = sum_{t<s} (1-A)A^(s-1-t) mu[(t,c)] + A^s m_state[c]
    v_vals[(s,c)] = sum_{t<s} (1-A)A^(s-1-t) w'[(t,c)] + A^s v_state[c]
      with w' = var + A*(mu-m)^2  (the (1-A) lives in the matrices)
    state'[c] = sum_t (1-A)A^(G-1-t) mu[(t,c)] + A^G state[c]
      (tail matrices replicate state' across all 4 sample slots)
    """
    A = AFWD
    k = np.zeros((P, CONST_COLS), np.float32)
    for s in range(G):
        for t in range(s):
            coef = (1 - A) * A ** (s - 1 - t)
            for c in range(CSH):
                k[t * CSH + c, COL_SCAN_M + s * CSH + c] = coef
                k[t * CSH + c, COL_SCAN_V + s * CSH + c] = coef
    for t in range(G):
        coef = (1 - A) * A ** (G - 1 - t)
        for s in range(G):
            for c in range(CSH):
                k[t * CSH + c, COL_TAIL_M + s * CSH + c] = coef
                k[t * CSH + c, COL_TAIL_V + s * CSH + c] = coef
    for s in range(G):
        k[s * CSH:(s + 1) * CSH, COL_APOW] = A ** s
    return k


_CACHE = {}


def build_nc(spt=SPT, xbufs=XBUFS, store_split=2):
    """Build (and cache) the Bass program. Same program for all 8 cores."""
    key = (spt, xbufs, store_split)
    if key in _CACHE:
        return _CACHE[key]
    nq = spt // G
    nt = N // spt

    import concourse.bacc as bacc
    import concourse.tile as tile
    from concourse import mybir

    f32 = mybir.dt.float32
    Alu = mybir.AluOpType
    Act = mybir.ActivationFunctionType
    AG = AFWD ** G

    nc = bacc.Bacc()
    x_d = nc.declare_dram_parameter("x", [N * CSH, FD], f32, isOutput=False)
    const_d = nc.declare_dram_parameter("consts", [P, CONST_COLS], f32,
                                        isOutput=False)
    out_d = nc.declare_dram_parameter("out", [N * CSH, FD], f32, isOutput=True)

    with tile.TileContext(nc) as tc, ExitStack() as ctx:
        const = ctx.enter_context(tc.tile_pool(name="const", bufs=1))
        xp = ctx.enter_context(tc.tile_pool(name="xp", bufs=xbufs))
        st = ctx.enter_context(tc.tile_pool(name="st", bufs=3))
        states = ctx.enter_context(tc.tile_pool(name="states", bufs=2))
        psA = ctx.enter_context(tc.tile_pool(name="psA", bufs=2, space="PSUM"))
        psB = ctx.enter_context(tc.tile_pool(name="psB", bufs=1, space="PSUM"))

        ct = const.tile([P, CONST_COLS], f32)
        nc.sync.dma_start(out=ct, in_=const_d[:])
        lhs_scan_m = ct[:, COL_SCAN_M:COL_SCAN_M + P]
        lhs_scan_v = ct[:, COL_SCAN_V:COL_SCAN_V + P]
        lhs_tail_m = ct[:, COL_TAIL_M:COL_TAIL_M + P]
        lhs_tail_v = ct[:, COL_TAIL_V:COL_TAIL_V + P]
        apow = ct[:, COL_APOW:COL_APOW + 1]

        # PE touches the const tile once, so later matmuls carry no DMA wait.
        warm = psB.tile([P, 1], f32)
        nc.tensor.matmul(warm, lhsT=lhs_scan_m, rhs=apow, start=True, stop=True)

        # replicated per-(s,c) carry state: every sample slot holds state[c]
        m_rep = states.tile([P, 1], f32)
        nc.vector.memset(m_rep, 0.0)
        v_rep = states.tile([P, 1], f32)
        nc.vector.memset(v_rep, 1.0)

        for g in range(nt):
            xt = xp.tile([P, nq * FD], f32)
            rows = slice(g * nq * P, (g + 1) * nq * P)
            if nq > 1:
                nc.sync.dma_start(
                    out=xt.rearrange("p (h f) -> p h f", h=nq),
                    in_=x_d[rows, :].rearrange("(h p) f -> p h f", p=P),
                )
            else:
                nc.sync.dma_start(out=xt, in_=x_d[rows, :])

            for q in range(nq):
                xq = xt[:, q * FD:(q + 1) * FD]

                # per-(sample,channel) mean/var over the 4096 free elements
                bnst = st.tile([P, FD // 512, 6], f32)
                xq_chunks = xq.rearrange("p (k f) -> p k f", f=512)
                for k in range(FD // 512):
                    nc.vector.bn_stats(out=bnst[:, k, :], in_=xq_chunks[:, k, :])
                mv = st.tile([P, 2], f32)
                nc.vector.bn_aggr(out=mv, in_=bnst)
                mu = mv[:, 0:1]
                var = mv[:, 1:2]

                # m_vals[(s,c)] = m_{n0+s,c}: triangular on PE, carry on DVE
                pm = psA.tile([P, 1], f32)
                nc.tensor.matmul(pm, lhsT=lhs_scan_m, rhs=mu, start=True,
                                 stop=True)
                pmrep = psB.tile([P, 1], f32)
                nc.tensor.matmul(pmrep, lhsT=lhs_tail_m, rhs=mu, start=True,
                                 stop=True)
                mc = st.tile([P, 1], f32)
                nc.vector.tensor_tensor(out=mc, in0=apow, in1=m_rep,
                                        op=Alu.mult)
                m_neg = st.tile([P, 1], f32)
                nc.vector.scalar_tensor_tensor(
                    out=m_neg, in0=pm, scalar=-1.0, in1=mc,
                    op0=Alu.mult, op1=Alu.subtract,
                )  # -(pm + A^s*state)

                # w' = var + A*(mu - m)^2
                d = st.tile([P, 1], f32)
                nc.vector.tensor_tensor(out=d, in0=mu, in1=m_neg, op=Alu.add)
                d2 = st.tile([P, 1], f32)
                nc.vector.tensor_tensor(out=d2, in0=d, in1=d, op=Alu.mult)
                wp = st.tile([P, 1], f32)
                nc.vector.scalar_tensor_tensor(
                    out=wp, in0=d2, scalar=AFWD, in1=var,
                    op0=Alu.mult, op1=Alu.add,
                )

                # v_vals + eps, assembled straight into SBUF
                pv = psA.tile([P, 1], f32)
                nc.tensor.matmul(pv, lhsT=lhs_scan_v, rhs=wp, start=True,
                                 stop=True)
                pvrep = psB.tile([P, 1], f32)
                nc.tensor.matmul(pvrep, lhsT=lhs_tail_v, rhs=wp, start=True,
                                 stop=True)
                vc = st.tile([P, 1], f32)
                nc.vector.tensor_tensor(out=vc, in0=apow, in1=v_rep,
                                        op=Alu.mult)
                ve = st.tile([P, 1], f32)
                nc.vector.scalar_tensor_tensor(
                    out=ve, in0=pv, scalar=EPS, in1=vc,
                    op0=Alu.add, op1=Alu.add,
                )  # pv + eps + A^s*v_state

                # next-quarter replicated states (serial chain)
                new_m = states.tile([P, 1], f32)
                nc.vector.scalar_tensor_tensor(
                    out=new_m, in0=m_rep, scalar=AG, in1=pmrep,
                    op0=Alu.mult, op1=Alu.add,
                )
                m_rep = new_m
                new_v = states.tile([P, 1], f32)
                nc.vector.scalar_tensor_tensor(
                    out=new_v, in0=v_rep, scalar=AG, in1=pvrep,
                    op0=Alu.mult, op1=Alu.add,
                )
                v_rep = new_v

                # scale = 1/sqrt(v + eps); bias = -m * scale
                s0 = st.tile([P, 1], f32)
                nc.scalar.activation(out=s0, in_=ve, func=Act.Sqrt)
                sc = st.tile([P, 1], f32)
                nc.vector.reciprocal(out=sc, in_=s0)
                b = st.tile([P, 1], f32)
                nc.vector.tensor_scalar(
                    out=b, in0=m_neg, scalar1=sc, scalar2=None, op0=Alu.mult
                )

                # out = x*scale + bias, in place
                nc.scalar.activation(
                    out=xq, in_=xq, func=Act.Identity, bias=b, scale=sc
                )

            # store on the ACT HWDGE ring, optionally in store_split pieces
            hs = nq // store_split
            for piece in range(store_split):
                prows = slice((g * nq + piece * hs) * P,
                              (g * nq + (piece + 1) * hs) * P)
                pxt = xt[:, piece * hs * FD:(piece + 1) * hs * FD]
                if hs > 1:
                    nc.scalar.dma_start(
                        out=out_d[prows, :].rearrange("(h p) f -> p h f", p=P),
                        in_=pxt.rearrange("p (h f) -> p h f", h=hs),
                    )
                else:
                    nc.scalar.dma_start(out=out_d[prows, :], in_=pxt)

    nc.compile()
    _CACHE[key] = nc
    return nc


def kernel(x: np.ndarray) -> np.ndarray:
    assert x.shape == (N, C, H, W) and x.dtype == np.float32
    nc = build_nc()
    from concourse.bass_utils import run_bass_kernel_spmd

    consts = _build_const()
    in_maps = []
    for k in range(NCORES):
        shard = np.ascontiguousarray(
            x[:, k * CSH:(k + 1) * CSH]
        ).reshape(N * CSH, FD)
        in_maps.append({"x": shard, "consts": consts})

    res = run_bass_kernel_spmd(nc, in_maps, core_ids=list(range(NCORES)))
    shards = [res.results[k]["out"].reshape(N, CSH, H, W) for k in range(NCORES)]
    return np.concatenate(shards, axis=1)


# revision 12
# speedup vs baseline: 119.8741x; 4.5469x over previous
"""ControlNorm2DLoop Trainium2 kernel.

x: [64, 256, 64, 64] f32. Per-(n,c) spatial moments over (H,W), then a
sequential EMA over the batch dim updates per-channel (m, v); each sample is
normalized with the state *before* its update.

Strategy: shard C across 8 cores (32 ch/core). Single pass over x in
supertiles of SPT samples -> SBUF tiles [128 = 4*32 partitions, NQ*4096],
where each quarter (4 samples x 32 channels) fills the 128 partitions and
quarters sit side by side in the free dim. Stats via bn_stats/bn_aggr (DVE).
The EMA recurrence is linear, so the within-quarter state propagation is a
constant triangular matrix applied on the TensorEngine (contracts over
partitions); the cross-quarter carry is a replicated [128,1] state tile
updated with elementwise DVE ops. Normalization is done in-place on the x
tile by the scalar engine (Identity(x*scale+bias) with per-partition
scale/bias). Loads are issued on the SP HWDGE ring, stores on the ACT HWDGE
ring.

PE wait discipline: walrus allows only ONE sync-wait command on a
(self-loading fp32) Matmult, so all constants arrive in a single DMA that a
warmup matmul observes once, and everything else a matmul touches (rhs
vectors, recycled PSUM slots) is produced/consumed exclusively by the DVE.
"""

import sys

if "/opt/trn_rl_repo" not in sys.path:
    sys.path.insert(0, "/opt/trn_rl_repo")

from contextlib import ExitStack

import numpy as np

AFWD = 0.999
EPS = 1e-05
N, C, H, W = 64, 256, 64, 64
NCORES = 8
CSH = C // NCORES     # 32 channels per core
G = 4                 # samples per quarter (fills 128 partitions)
FD = H * W            # 4096
P = G * CSH           # 128 partitions

SPT = 8               # samples per supertile
NQ = SPT // G         # quarters per supertile
NT = N // SPT         # supertiles per core
XBUFS = 5             # supertile double/triple buffering

# packed const layout (columns of the [128, 513] const tile)
COL_SCAN_M = 0
COL_SCAN_V = 128
COL_TAIL_M = 256
COL_TAIL_V = 384
COL_APOW = 512
CONST_COLS = 513


def _build_const() -> np.ndarray:
    """One [128, 513] f32 tile holding all scan matrices + A^s column.

    m_vals[(s,c)] = sum_{t<s} (1-A)A^(s-1-t) mu[(t,c)] + A^s m_state[c]
    v_vals[(s,c)] = sum_{t<s} (1-A)A^(s-1-t) w'[(t,c)] + A^s v_state[c]
      with w' = var + A*(mu-m)^2  (the (1-A) lives in the matrices)
    state'[c] = sum_t (1-A)A^(G-1-t) mu[(t,c)] + A^G state[c]
      (tail matrices replicate state' across all 4 sample slots)
    """
    A = AFWD
    k = np.zeros((P, CONST_COLS), np.float32)
    for s in range(G):
        for t in range(s):
            coef = (1 - A) * A ** (s - 1 - t)
            for c in range(CSH):
                k[t * CSH + c, COL_SCAN_M + s * CSH + c] = coef
                k[t * CSH + c, COL_SCAN_V + s * CSH + c] = coef
    for t in range(G):
        coef = (1 - A) * A ** (G - 1 - t)
        for s in range(G):
            for c in range(CSH):
                k[t * CSH + c, COL_TAIL_M + s * CSH + c] = coef
                k[t * CSH + c, COL_TAIL_V + s * CSH + c] = coef
    for s in range(G):
        k[s * CSH:(s + 1) * CSH, COL_APOW] = A ** s
    return k


_CACHE = {}


def build_nc(spt=SPT, xbufs=XBUFS, store_split=2):
    """Build (and cache) the Bass program. Same program for all 8 cores."""
    key = (spt, xbufs, store_split)
    if key in _CACHE:
        return _CACHE[key]
    nq = spt // G
    nt = N // spt

    import concourse.bacc as bacc
    import concourse.tile as tile
    from concourse import mybir

    f32 = mybir.dt.float32
    Alu = mybir.AluOpType
    Act = mybir.ActivationFunctionType
    AG = AFWD ** G

    nc = bacc.Bacc()
    x_d = nc.declare_dram_parameter("x", [N * CSH, FD], f32, isOutput=False)
    const_d = nc.declare_dram_parameter("consts", [P, CONST_COLS], f32,
                                        isOutput=False)
    out_d = nc.declare_dram_parameter("out", [N * CSH, FD], f32, isOutput=True)

    with tile.TileContext(nc) as tc, ExitStack() as ctx:
        const = ctx.enter_context(tc.tile_pool(name="const", bufs=1))
        xp = ctx.enter_context(tc.tile_pool(name="xp", bufs=xbufs))
        st = ctx.enter_context(tc.tile_pool(name="st", bufs=3))
        states = ctx.enter_context(tc.tile_pool(name="states", bufs=2))
        psA = ctx.enter_context(tc.tile_pool(name="psA", bufs=2, space="PSUM"))
        psB = ctx.enter_context(tc.tile_pool(name="psB", bufs=1, space="PSUM"))

        ct = const.tile([P, CONST_COLS], f32)
        nc.sync.dma_start(out=ct, in_=const_d[:])
        lhs_scan_m = ct[:, COL_SCAN_M:COL_SCAN_M + P]
        lhs_scan_v = ct[:, COL_SCAN_V:COL_SCAN_V + P]
        lhs_tail_m = ct[:, COL_TAIL_M:COL_TAIL_M + P]
        lhs_tail_v = ct[:, COL_TAIL_V:COL_TAIL_V + P]
        apow = ct[:, COL_APOW:COL_APOW + 1]

        # PE touches the const tile once, so later matmuls carry no DMA wait.
        warm = psB.tile([P, 1], f32)
        nc.tensor.matmul(warm, lhsT=lhs_scan_m, rhs=apow, start=True, stop=True)

        # replicated per-(s,c) carry state: every sample slot holds state[c]
        m_rep = states.tile([P, 1], f32)
        nc.vector.memset(m_rep, 0.0)
        v_rep = states.tile([P, 1], f32)
        nc.vector.memset(v_rep, 1.0)

        for g in range(nt):
            xt = xp.tile([P, nq * FD], f32)
            rows = slice(g * nq * P, (g + 1) * nq * P)
            if nq > 1:
                nc.sync.dma_start(
                    out=xt.rearrange("p (h f) -> p h f", h=nq),
                    in_=x_d[rows, :].rearrange("(h p) f -> p h f", p=P),
                )
            else:
                nc.sync.dma_start(out=xt, in_=x_d[rows, :])

            for q in range(nq):
                xq = xt[:, q * FD:(q + 1) * FD]

                # per-(sample,channel) mean/var over the 4096 free elements
                bnst = st.tile([P, FD // 512, 6], f32)
                xq_chunks = xq.rearrange("p (k f) -> p k f", f=512)
                for k in range(FD // 512):
                    nc.vector.bn_stats(out=bnst[:, k, :], in_=xq_chunks[:, k, :])
                mv = st.tile([P, 2], f32)
                nc.vector.bn_aggr(out=mv, in_=bnst)
                mu = mv[:, 0:1]
                var = mv[:, 1:2]

                # m_vals[(s,c)] = m_{n0+s,c}: triangular on PE, carry on DVE
                pm = psA.tile([P, 1], f32)
                nc.tensor.matmul(pm, lhsT=lhs_scan_m, rhs=mu, start=True,
                                 stop=True)
                pmrep = psB.tile([P, 1], f32)
                nc.tensor.matmul(pmrep, lhsT=lhs_tail_m, rhs=mu, start=True,
                                 stop=True)
                mc = st.tile([P, 1], f32)
                nc.vector.tensor_tensor(out=mc, in0=apow, in1=m_rep,
                                        op=Alu.mult)
                m_neg = st.tile([P, 1], f32)
                nc.vector.scalar_tensor_tensor(
                    out=m_neg, in0=pm, scalar=-1.0, in1=mc,
                    op0=Alu.mult, op1=Alu.subtract,
                )  # -(pm + A^s*state)

                # w' = var + A*(mu - m)^2
                d = st.tile([P, 1], f32)
                nc.vector.tensor_tensor(out=d, in0=mu, in1=m_neg, op=Alu.add)
                d2 = st.tile([P, 1], f32)
                nc.vector.tensor_tensor(out=d2, in0=d, in1=d, op=Alu.mult)
                wp = st.tile([P, 1], f32)
                nc.vector.scalar_tensor_tensor(
                    out=wp, in0=d2, scalar=AFWD, in1=var,
                    op0=Alu.mult, op1=Alu.add,
                )

                # v_vals + eps, assembled straight into SBUF
                pv = psA.tile([P, 1], f32)
                nc.tensor.matmul(pv, lhsT=lhs_scan_v, rhs=wp, start=True,
                                 stop=True)
                pvrep = psB.tile([P, 1], f32)
                nc.tensor.matmul(pvrep, lhsT=lhs_tail_v, rhs=wp, start=True,
                                 stop=True)
                vc = st.tile([P, 1], f32)
                nc.vector.tensor_tensor(out=vc, in0=apow, in1=v_rep,
                                        op=Alu.mult)
                ve = st.tile([P, 1], f32)
                nc.vector.scalar_tensor_tensor(
                    out=ve, in0=pv, scalar=EPS, in1=vc,
                    op0=Alu.add, op1=Alu.add,
                )  # pv + eps + A^s*v_state

                # next-quarter replicated states (serial chain)
                new_m = states.tile([P, 1], f32)
                nc.vector.scalar_tensor_tensor(
                    out=new_m, in0=m_rep, scalar=AG, in1=pmrep,
                    op0=Alu.mult, op1=Alu.add,
                )
                m_rep = new_m
                new_v = states.tile([P, 1], f32)
                nc.vector.scalar_tensor_tensor(
                    out=new_v, in0=v_rep, scalar=AG, in1=pvrep,
                    op0=Alu.mult, op1=Alu.add,
                )
                v_rep = new_v

                # scale = 1/sqrt(v + eps); bias = -m * scale
                s0 = st.tile([P, 1], f32)
                nc.scalar.activation(out=s0, in_=ve, func=Act.Sqrt)
                sc = st.tile([P, 1], f32)
                nc.vector.reciprocal(out=sc, in_=s0)
                b = st.tile([P, 1], f32)
                nc.vector.tensor_scalar(
                    out=b, in0=m_neg, scalar1=sc, scalar2=None, op0=Alu.mult
                )

                # out = x*scale + bias, in place
                nc.scalar.activation(
                    out=xq, in_=xq, func=Act.Identity, bias=b, scale=sc
                )

            # store on the ACT HWDGE ring, optionally in store_split pieces
            hs = nq // store_split
            for piece in range(store_split):
                prows = slice((g * nq + piece * hs) * P,
                              (g * nq + (piece + 1) * hs) * P)
                pxt = xt[:, piece * hs * FD:(piece + 1) * hs * FD]
                if hs > 1:
                    nc.scalar.dma_start(
                        out=out_d[prows, :].rearrange("(h p) f -> p h f", p=P),
                        in_=pxt.rearrange("p (h f) -> p h f", h=hs),
                    )
                else:
                    nc.scalar.dma_start(out=out_d[prows, :], in_=pxt)

    nc.compile()
    _CACHE[key] = nc
    return nc


def kernel(x) -> np.ndarray:
    x = np.asarray(x, dtype=np.float32)
    assert x.shape == (N, C, H, W), x.shape
    nc = build_nc()
    from concourse.bass_utils import run_bass_kernel_spmd

    consts = _build_const()
    in_maps = []
    for k in range(NCORES):
        shard = np.ascontiguousarray(
            x[:, k * CSH:(k + 1) * CSH]
        ).reshape(N * CSH, FD)
        in_maps.append({"x": shard, "consts": consts})

    res = run_bass_kernel_spmd(nc, in_maps, core_ids=list(range(NCORES)))
    shards = [res.results[k]["out"].reshape(N, CSH, H, W) for k in range(NCORES)]
    return np.concatenate(shards, axis=1)
